# revision 34
# baseline (speedup 1.0000x reference)
"""Single-launch batch-split attention kernel: projections + biased softmax
attention + normalization + output projection on-device, one SPMD dispatch.

Sharding: core c owns batch b=c, all 16 heads (loop over 8 head-pairs hp).
Per hp the transposed-logits pipeline computes LT[t,s] = K @ Q^T with 4-way
PE row-tiling (contraction 32/head), exp, multiplicative rel-pos bias, and
PV with an appended ones column so the softmax denominator Z rides along.

Key structural points vs the naive version:
- The [S,S] multiplicative bias exp(rel_bias) is block-Toeplitz (32x32
  blocks depending only on h_s - h_t), so instead of streaming the dense
  33.5MB exp-bias per core we stream an 8.25MB per-partition PRE-SHIFTED
  compact table tab[p=(bi,wt), h, x] = exp(rb[n, x//32 - bi, x%32 - wt + 31]).
  The shift baked into each partition's content makes the bias operand of
  every probs multiply a plain contiguous slice tab[:, :, off:off+512] with
  off = (31 - 4*tj + 16*sc)*32 — same DVE cost as a dense tile.
- Engine balance: SCH_SET tiles compute probs in ONE VectorE op via a
  bias-fused fp16 Schraudolph (int16(lt*A + tabb) bitcast fp16 — the bias
  ADD rides the affine, no separate multiply); the rest use ScalarE exp +
  a bf16 bias multiply on VectorE or GpSimd/Pool (MULT_POOL_SET). The
  PSUM->SBUF moves are spread (qk add, au copy, out copy on ScalarE; v_b
  on VectorE with persistent ones/zero columns).
- Normalization is distributed per-hp: each hp's 1/Z reciprocal + K=4
  selection-matmul broadcast + normalize + atf scatter are emitted under
  the NEXT hp's compute, so only hp7's norm remains in the tail.
- DMA issue (previously ~1.2us serial per DMA on the SP queue) is cut by
  the compact tables, merged output stores, and deferred zc-gather
  emission so prefetches are never stuck behind data-dependent DMAs.
"""

import os

import numpy as np

import concourse.bass as bass
from concourse import bacc
import concourse.mybir as mybir
import concourse.tile as tile
from concourse.bass_utils import run_bass_kernel_spmd

B, S, D = 8, 1024, 512
NH, KD = 16, 32
H = W = 32
P = 128
NCORES = 8
HPC = 2                   # heads per hp-group
NHP = NH // HPC           # 8 head-pair groups
TABX = 63 * 32            # compact bias table free extent (2016)
F32 = mybir.dt.float32
BF16 = mybir.dt.float16
I16 = mybir.dt.int16

# (sc, tj) tiles whose exp runs as a bias-fused fp16 Schraudolph on VectorE:
# probs = bitcast_fp16(int16(lt*A + tabb)), tabb = round(A*rb + B) — one op,
# no separate bias multiply. Remaining tiles: exp on ScalarE + bias multiply
# on Pool (MULT_POOL_SET) or VectorE.
SCH_SET = frozenset(
    [(0, 0), (0, 3), (0, 5), (1, 1), (1, 3), (1, 4), (1, 6)]
)
MULT_POOL_SET = frozenset(
    [(0, 4), (1, 1), (1, 6)]
)
# fp16 Schraudolph: exp(x) ~= bitcast_fp16(int16(A*x + Bc))
SCH_A = float(1024.0 / np.log(2.0))
SCH_B = 15316.0

LAST_RESULTS = []
LAST_INMAPS = None


def _build_merged(repeat=1):
    nc = bacc.Bacc()
    qtb = nc.declare_dram_parameter("qtb", [D, S], BF16, isOutput=False)
    wqk = nc.declare_dram_parameter("wqk", [D, NHP * P], BF16, isOutput=False)
    wv = nc.declare_dram_parameter("wv", [D, NHP * HPC * KD], BF16, isOutput=False)
    bqk = nc.declare_dram_parameter("bqk", [P, NHP], F32, isOutput=False)
    sel = nc.declare_dram_parameter("sel", [4, 2 * P], BF16, isOutput=False)
    wo = nc.declare_dram_parameter("wo", [NH * KD, D], BF16, isOutput=False)
    # compact shifted bias tables: [p, hp, h, x]; tab = exp(rb) (bf16, for
    # the ScalarE-exp path's multiply), tabb = round(A*rb + B) (int16, the
    # fused addend for the Schraudolph path)
    tab = nc.declare_dram_parameter(
        "tab", [P, NHP * HPC * TABX], BF16, isOutput=False
    )
    tabb = nc.declare_dram_parameter(
        "tabb", [P, NHP * HPC * TABX], I16, isOutput=False
    )
    o = nc.declare_dram_parameter("o", [S, D], F32, isOutput=True)

    tab_r = tab.rearrange("p (hp h x) -> p hp h x", hp=NHP, h=HPC)
    tabb_r = tabb.rearrange("p (hp h x) -> p hp h x", hp=NHP, h=HPC)

    with tile.TileContext(nc) as tc:
        with (
            tc.tile_pool(name="const", bufs=1) as cpool,
            tc.tile_pool(name="qkp", bufs=2) as qkpool,
            tc.tile_pool(name="vp", bufs=2) as vpool,
            tc.tile_pool(name="repp", bufs=2) as reppool,
            tc.tile_pool(name="ebp", bufs=2) as ebpool,
            tc.tile_pool(name="expp", bufs=6) as exppool,
            tc.tile_pool(name="probsp", bufs=6) as probspool,
            tc.tile_pool(name="attp", bufs=16) as attpool,
            tc.tile_pool(name="zcp", bufs=1) as zcpool,
            tc.tile_pool(name="atnp", bufs=4) as atnpool,
            tc.tile_pool(name="atfp", bufs=1) as atfpool,
            tc.tile_pool(name="outp", bufs=2) as opool,
        ):
            # ---- constants ----
            qtb_t = cpool.tile([P, 4, S], BF16, name="qtb_t")
            nc.sync.dma_start(qtb_t, qtb.rearrange("(c p) s -> p c s", p=P))
            wqk_sb = cpool.tile([P, 4, NHP, P], BF16, name="wqk_sb")
            nc.sync.dma_start(
                wqk_sb, wqk.rearrange("(c p) (hp m) -> p c hp m", p=P, hp=NHP)
            )
            wv_sb = cpool.tile([P, 4, NHP, HPC * KD], BF16, name="wv_sb")
            nc.sync.dma_start(
                wv_sb, wv.rearrange("(c p) (hp m) -> p c hp m", p=P, hp=NHP)
            )
            bqk_sb = cpool.tile([P, NHP], F32, name="bqk_sb")
            nc.sync.dma_start(bqk_sb, bqk[:, :])
            sel_sb = cpool.tile([P, 2, P], BF16, name="sel_sb")
            wo_sb = cpool.tile([P, 4, D], BF16, name="wo_sb")
            tail_consts = []

            def load_tail_consts():
                # deferred so these DMAs don't compete with the startup loads
                if not tail_consts:
                    nc.sync.dma_start(
                        sel_sb[0:4], sel.rearrange("r (i p) -> r i p", i=2)
                    )
                    nc.sync.dma_start(wo_sb, wo.rearrange("(c p) d -> p c d", p=P))
                    tail_consts.append(True)

            zc_hp = {}

            # persistent 64-wide v tiles (double-buffered across hp):
            # cols 0:32 = V (rewritten per hp), col 32 = ones (Z rides
            # along), cols 33:64 = zeros so PV writes every atps row --
            # constant columns initialized once.
            v_bs = [
                vpool.tile([P, 8, HPC, 64], BF16, name=f"v_b{i}", tag=f"v_b{i}")
                for i in range(2)
            ]
            for vb in v_bs:
                nc.vector.memset(vb[:, :, :, KD + 1 :], 0.0)
                nc.vector.memset(vb[:, :, :, KD : KD + 1], 1.0)

            def emit_logits(nsp, g, rep, qk_b, tsl, ssl):
                for h in range(HPC):
                    if g == 0:
                        nc.tensor.matmul(
                            nsp["lt"][g][:, h * 512 : (h + 1) * 512],
                            lhsT=rep[h * KD : (h + 1) * KD, tsl],
                            rhs=qk_b[h * KD : (h + 1) * KD, ssl],
                            start=True,
                            stop=True,
                            tile_position=(h * KD, 0),
                        )
                    else:
                        nc.tensor.matmul(
                            nsp["lt"][g][:, h * 512 : (h + 1) * 512],
                            lhsT=qk_b[64 + h * KD : 64 + (h + 1) * KD, tsl],
                            rhs=rep[64 + h * KD : 64 + (h + 1) * KD, ssl],
                            start=True,
                            stop=True,
                            tile_position=(64 + h * KD, 0),
                        )

            def emit_sc(nsp, hp, sc, qk_b, rep, v_b, tab_t, tabb_t):
                ssl = slice(sc * 512, (sc + 1) * 512)
                atps = nsp["psattn"].tile([P, 512], F32, name="atps", tag="atps")
                for tj in range(8):
                    g = tj % 2
                    tsl = slice(tj * P, (tj + 1) * P)
                    with tc.high_priority(offset=64):
                        emit_logits(nsp, g, rep, qk_b, tsl, ssl)
                    ltg = nsp["lt"][g]
                    probs = probspool.tile([P, 1024], BF16, name="probs", tag="probs")
                    off = (31 - 4 * tj + 16 * sc) * 32
                    if (sc, tj) in SCH_SET:
                        # bias-fused Schraudolph: probs = fp16^(lt*A + tabb)
                        nc.vector.scalar_tensor_tensor(
                            probs.bitcast(I16).rearrange("p (h s) -> p h s", h=HPC),
                            ltg.rearrange("p (h s) -> p h s", h=HPC),
                            SCH_A,
                            tabb_t[:, :, off : off + 512],
                            mybir.AluOpType.mult,
                            mybir.AluOpType.add,
                        )
                    else:
                        exp_t = exppool.tile(
                            [P, 1024], BF16, name="exp_t", tag="exp_t"
                        )
                        nc.scalar.activation(
                            exp_t, ltg, mybir.ActivationFunctionType.Exp
                        )
                        mult_eng = (
                            nc.gpsimd if (sc, tj) in MULT_POOL_SET else nc.vector
                        )
                        mult_eng.tensor_tensor(
                            probs.rearrange("p (h s) -> p h s", h=HPC),
                            exp_t.rearrange("p (h s) -> p h s", h=HPC),
                            tab_t[:, :, off : off + 512],
                            mybir.AluOpType.mult,
                        )
                    for h in range(HPC):
                        nc.tensor.matmul(
                            atps[h * 64 : h * 64 + 64, :],
                            lhsT=v_b[:, tj, h, :],
                            rhs=probs[:, h * 512 : (h + 1) * 512],
                            start=(tj == 0),
                            stop=(tj == 7),
                            tile_position=(0, h * 64),
                            skip_group_check=True,
                        )
                au = attpool.tile([P, 512], BF16, name="au", tag="au")
                nc.scalar.copy(au, atps)
                pending_zc.append((hp, sc, au))
                return au

            pending_zc = []

            def flush_zc():
                while pending_zc:
                    hp_, sc_, au = pending_zc.pop(0)
                    if hp_ not in zc_hp:
                        zc_hp[hp_] = zcpool.tile(
                            [4, 512], BF16, name=f"zc{hp_}", tag="zc", bufs=3
                        )
                    z = zc_hp[hp_]
                    nc.sync.dma_start(z[2 * sc_ : 2 * sc_ + 1, :], au[32:33, :])
                    nc.sync.dma_start(
                        z[2 * sc_ + 1 : 2 * sc_ + 2, :], au[96:97, :]
                    )

            def emit_hp(nsp, hp, att_un, mid=None):
                flush_zc()
                qk_b = qkpool.tile([P, S], BF16, name="qk_b", tag="qk_b")
                v_b = v_bs[hp % 2]
                for sc in range(2):
                    ssl = slice(sc * 512, (sc + 1) * 512)
                    qkps = nsp["ps1"].tile(
                        [P, 512], F32, name="qkps", tag="proj", bufs=2
                    )
                    for ch in range(4):
                        nc.tensor.matmul(
                            qkps,
                            lhsT=wqk_sb[:, ch, hp, :],
                            rhs=qtb_t[:, ch, ssl],
                            start=(ch == 0),
                            stop=(ch == 3),
                        )
                    nc.scalar.add(qk_b[:, ssl], qkps, bqk_sb[:, hp : hp + 1])
                vps = nsp["ps1"].tile([P, 512], F32, name="vps", tag="proj", bufs=2)
                for tj in range(8):
                    for ch in range(4):
                        nc.tensor.matmul(
                            vps[:, tj * 64 : (tj + 1) * 64],
                            lhsT=qtb_t[:, ch, tj * P : (tj + 1) * P],
                            rhs=wv_sb[:, ch, hp, :],
                            start=(ch == 0),
                            stop=(ch == 3),
                        )
                nc.vector.tensor_copy(
                    v_b[:, :, :, 0:KD],
                    vps.rearrange("p (tj h k) -> p tj h k", tj=8, h=HPC),
                )
                rep = reppool.tile([P, S], BF16, name="rep", tag="rep")
                nc.sync.dma_start(rep[0:64, :], qk_b[64:128, :])
                nc.sync.dma_start(rep[64:128, :], qk_b[0:64, :])
                tab_t = ebpool.tile([P, HPC, TABX], BF16, name="tab_t", tag="tab_t")
                nc.sync.dma_start(tab_t, tab_r[:, hp])
                tabb_t = ebpool.tile(
                    [P, HPC, TABX], I16, name="tabb_t", tag="tabb_t"
                )
                nc.sync.dma_start(tabb_t, tabb_r[:, hp])
                if mid is not None:
                    mid()
                for sc in range(2):
                    att_un[(hp, sc)] = emit_sc(
                        nsp, hp, sc, qk_b, rep, v_b, tab_t, tabb_t
                    )

            def emit_norm_hp(hp, att_un, atf, rz_unused, rzpool):
                # 1/Z for this hp's 4 Z rows on VectorE, then per sc:
                # partition broadcast via a K=4 selection matmul, normalize,
                # scatter into the atf chunk for this hp.
                load_tail_consts()
                rz = zcpool.tile([4, 512], BF16, name=f"rz{hp}", tag="rz", bufs=2)
                with nc.allow_low_precision(reason="1/Z in fp16 (~5e-4)"):
                    nc.vector.reciprocal(rz[0:4, :], zc_hp[hp][0:4, :])
                for sc in range(2):
                    rzb = rzpool.tile([P, 512], F32, name="rzb", tag="rzb")
                    nc.tensor.matmul(
                        rzb,
                        lhsT=sel_sb[0:4, sc, :],
                        rhs=rz[0:4, :],
                        start=True,
                        stop=True,
                    )
                    au = att_un[(hp, sc)]
                    atn = atnpool.tile([P, 512], BF16, name="atn", tag="atn")
                    nc.vector.tensor_tensor(atn, au, rzb, mybir.AluOpType.mult)
                    # heads (2hp, 2hp+1) -> atf[hp%4] rows 64*(hp//4)+{0,32}
                    q, gq = hp % 4, hp // 4
                    for h in range(HPC):
                        nc.sync.dma_start(
                            atf[q][
                                64 * gq + 32 * h : 64 * gq + 32 * h + 32,
                                sc * 512 : sc * 512 + 512,
                            ],
                            atn[64 * h : 64 * h + 32, :],
                        )

            def emit_proj(atf, pso):
                for oc in range(2):
                    o_sb = opool.tile([P, 4, D], F32, name="o_sb", tag="o_sb")
                    for stl in range(4):
                        st = oc * 4 + stl
                        ps_o = pso.tile([P, D], F32, name="ps_o", tag="ps_o")
                        for ch in range(4):
                            nc.tensor.matmul(
                                ps_o,
                                lhsT=atf[ch][:, st * P : (st + 1) * P],
                                rhs=wo_sb[:, ch, :],
                                start=(ch == 0),
                                stop=(ch == 3),
                            )
                        nc.scalar.copy(o_sb[:, stl, :], ps_o)
                    nc.sync.dma_start(
                        o.rearrange("(oc st p) d -> p oc st d", p=P, oc=2)[
                            :, oc, :, :
                        ],
                        o_sb,
                    )

            for _rep in range(repeat):
                ps1_cm = tc.tile_pool(name="ps1", bufs=1, space="PSUM")
                ltpool_cm = tc.tile_pool(name="ltpool", bufs=1, space="PSUM")
                psattn_cm = tc.tile_pool(name="psattn", bufs=1, space="PSUM")
                nsp = {
                    "ps1": ps1_cm.__enter__(),
                    "ltpool": ltpool_cm.__enter__(),
                    "psattn": psattn_cm.__enter__(),
                }
                nsp["lt"] = [
                    nsp["ltpool"].tile([P, 1024], F32, name=f"lt{g}", tag=f"lt{g}")
                    for g in range(2)
                ]
                atf = [
                    atfpool.tile([P, S], BF16, name=f"atf{q}", tag=f"atf{q}")
                    for q in range(4)
                ]

                rzpool_cm = tc.tile_pool(name="rzps", bufs=1, space="PSUM")
                rzpool = rzpool_cm.__enter__()
                att_un = {}
                for hp in range(NHP):
                    mid = None
                    if hp >= 1:
                        mid = (lambda hp_: lambda: emit_norm_hp(
                            hp_, att_un, atf, None, rzpool
                        ))(hp - 1)
                    emit_hp(nsp, hp, att_un, mid=mid)
                flush_zc()
                emit_norm_hp(7, att_un, atf, None, rzpool)
                rzpool_cm.__exit__(None, None, None)
                psattn_cm.__exit__(None, None, None)
                ltpool_cm.__exit__(None, None, None)
                ps1_cm.__exit__(None, None, None)
                pso_cm = tc.tile_pool(name="pso", bufs=4, space="PSUM")
                pso = pso_cm.__enter__()
                emit_proj(atf, pso)
                pso_cm.__exit__(None, None, None)
    nc.compile()
    return nc


_NC = None
_IDX = None
_PREP_CACHE = {}


def _fingerprint(*arrs):
    import zlib
    h = 0
    for a in arrs:
        c = np.ascontiguousarray(a)
        h = zlib.crc32(c.view(np.uint8).reshape(-1), h)
        h = zlib.crc32(repr((c.shape, c.dtype.str)).encode(), h)
    return h


def _prep_static(Wq, Wk, Wv, Wo, bq, bk, rel_bias):
    scale = np.float32(KD ** -0.5)
    wqk_a = np.empty((D, NHP, 4, KD), dtype=np.float16)
    wv_a = np.empty((D, NHP, HPC, KD), dtype=np.float16)
    bqk_a = np.empty((P, NHP), dtype=np.float32)
    for hp in range(NHP):
        n0, n1 = 2 * hp, 2 * hp + 1
        wqk_a[:, hp, 0] = Wq[:, n0] * scale
        wqk_a[:, hp, 1] = Wq[:, n1] * scale
        wqk_a[:, hp, 2] = Wk[:, n0]
        wqk_a[:, hp, 3] = Wk[:, n1]
        wv_a[:, hp, 0] = Wv[:, n0]
        wv_a[:, hp, 1] = Wv[:, n1]
        bqk_a[:, hp] = np.concatenate(
            [bq[n0] * scale, bq[n1] * scale, bk[n0], bk[n1]]
        )
    # compact shifted bias table: tab[p=(bi,wt), n, x] =
    #   exp(rb[n, x//32 - bi, (x%32) - wt + 31]) (0 where d out of range).
    # The probs multiply for (tj, sc) then reads the contiguous slice
    # x in [off, off+512), off = (31 - 4*tj + 16*sc)*32.
    erb = np.exp(rel_bias).astype(np.float32)  # [N, 63, 63]
    brb = np.round(SCH_A * rel_bias + SCH_B).astype(np.int16)
    tab_a = np.zeros((P, NH, 63, 32), dtype=np.float16)
    tabb_a = np.zeros((P, NH, 63, 32), dtype=np.int16)
    bi_ = np.arange(P) // 32          # [P]
    wt_ = np.arange(P) % 32           # [P]
    ws_ = np.arange(32)
    for xb in range(63):
        d = xb - bi_                  # [P]
        valid = (d >= 0) & (d <= 62)
        wi = ws_[None, :] - wt_[:, None] + 31   # [P, 32] in [0, 62]
        for n in range(NH):
            blk = erb[n][np.clip(d, 0, 62)[:, None], wi]  # [P, 32]
            blk[~valid] = 0.0
            tab_a[:, n, xb, :] = blk
            blkb = brb[n][np.clip(d, 0, 62)[:, None], wi]
            blkb[~valid] = 0
            tabb_a[:, n, xb, :] = blkb
    tab_a = tab_a.reshape(P, NHP * HPC * TABX)
    tabb_a = tabb_a.reshape(P, NHP * HPC * TABX)
    sel_a = np.zeros((4, 2, P), dtype=np.float16)
    for sc in range(2):
        sel_a[2 * sc, sc, 0:64] = 1.0
        sel_a[2 * sc + 1, sc, 64:128] = 1.0
    sel_a = sel_a.reshape(4, 2 * P)
    # wo rows: atf[q] partition (gq*64 + h*32 + k) <-> head n = 2*(4*gq+q)+h
    wo_a = np.empty((4, 2, 2, KD, D), dtype=np.float16)
    for q in range(4):
        for gq in range(2):
            for h in range(2):
                wo_a[q, gq, h] = Wo[2 * (4 * gq + q) + h]
    return dict(
        wqk=np.ascontiguousarray(wqk_a.reshape(D, NHP * P)),
        wv=np.ascontiguousarray(wv_a.reshape(D, NHP * HPC * KD)),
        bqk=bqk_a,
        sel=sel_a,
        wo=np.ascontiguousarray(wo_a.reshape(NH * KD, D)),
        tab=np.ascontiguousarray(tab_a),
        tabb=np.ascontiguousarray(tabb_a),
    )


def kernel(query, Wq, bq, Wk, bk, Wv, bv, Wo, bo, rel_bias):
    global _NC
    query = np.asarray(query, dtype=np.float32)
    Wq = np.asarray(Wq, dtype=np.float32)
    Wk = np.asarray(Wk, dtype=np.float32)
    Wv = np.asarray(Wv, dtype=np.float32)
    Wo = np.asarray(Wo, dtype=np.float32)
    bq = np.asarray(bq, dtype=np.float32)
    bk = np.asarray(bk, dtype=np.float32)
    bv = np.asarray(bv, dtype=np.float32)
    bo = np.asarray(bo, dtype=np.float32)
    rel_bias = np.asarray(rel_bias, dtype=np.float32)

    trace = bool(int(os.environ.get("ATTN_TRACE", "0")))
    core_ids = list(range(NCORES))

    wkey = _fingerprint(Wq, Wk, Wv, Wo, bq, bk, rel_bias)
    if wkey not in _PREP_CACHE:
        _PREP_CACHE[wkey] = _prep_static(Wq, Wk, Wv, Wo, bq, bk, rel_bias)
    static_map = _PREP_CACHE[wkey]

    qkey = _fingerprint(query)
    if qkey not in _PREP_CACHE:
        _PREP_CACHE[qkey] = [
            np.ascontiguousarray(query[c].T.astype(np.float16)) for c in range(NCORES)
        ]
    qtbs = _PREP_CACHE[qkey]

    in_maps = [dict(qtb=qtbs[c], **static_map) for c in range(NCORES)]
    global LAST_INMAPS
    LAST_INMAPS = in_maps
    if _NC is None:
        _NC = _build_merged()
    r = run_bass_kernel_spmd(_NC, in_maps, core_ids, trace=trace)
    LAST_RESULTS.clear()
    LAST_RESULTS.append(r)

    out = np.stack([r.results[c]["o"] for c in range(NCORES)])  # [B, S, D]
    bo_eff = bo + np.einsum("nk,nkd->d", bv, Wo)
    return (out + bo_eff[None, None, :]).astype(np.float32)


# revision 35
# speedup vs baseline: 1.0423x; 1.0423x over previous
"""Single-launch batch-split attention kernel: projections + biased softmax
attention + normalization + output projection on-device, one SPMD dispatch.

Sharding: core c owns batch b=c, all 16 heads (loop over 8 head-pairs hp).
Per hp the transposed-logits pipeline computes LT[t,s] = K @ Q^T with 4-way
PE row-tiling (contraction 32/head), exp, multiplicative rel-pos bias, and
PV with an appended ones column so the softmax denominator Z rides along.

Key structural points vs the naive version:
- The [S,S] multiplicative bias exp(rel_bias) is block-Toeplitz (32x32
  blocks depending only on h_s - h_t), so instead of streaming the dense
  33.5MB exp-bias per core we stream an 8.25MB per-partition PRE-SHIFTED
  compact table tab[p=(bi,wt), h, x] = exp(rb[n, x//32 - bi, x%32 - wt + 31]).
  The shift baked into each partition's content makes the bias operand of
  every probs multiply a plain contiguous slice tab[:, :, off:off+512] with
  off = (31 - 4*tj + 16*sc)*32 — same DVE cost as a dense tile.
- Engine balance: SCH_SET tiles compute probs in ONE VectorE op via a
  bias-fused fp16 Schraudolph (int16(lt*A + tabb) bitcast fp16 — the bias
  ADD rides the affine, no separate multiply); the rest use ScalarE exp +
  a bf16 bias multiply on VectorE or GpSimd/Pool (MULT_POOL_SET). The
  PSUM->SBUF moves are spread (qk add, au copy, out copy on ScalarE; v_b
  on VectorE with persistent ones/zero columns).
- Normalization is distributed per-hp: each hp's 1/Z reciprocal + K=4
  selection-matmul broadcast + normalize + atf scatter are emitted under
  the NEXT hp's compute, so only hp7's norm remains in the tail.
- DMA issue (previously ~1.2us serial per DMA on the SP queue) is cut by
  the compact tables, merged output stores, and deferred zc-gather
  emission so prefetches are never stuck behind data-dependent DMAs.
"""

import os

import numpy as np

import concourse.bass as bass
from concourse import bacc
import concourse.mybir as mybir
import concourse.tile as tile
from concourse.bass_utils import run_bass_kernel_spmd

B, S, D = 8, 1024, 512
NH, KD = 16, 32
H = W = 32
P = 128
NCORES = 8
HPC = 2                   # heads per hp-group
NHP = NH // HPC           # 8 head-pair groups
TABX = 63 * 32            # compact bias table free extent (2016)
F32 = mybir.dt.float32
BF16 = mybir.dt.float16
I16 = mybir.dt.int16

# (sc, tj) tiles whose exp runs as a bias-fused fp16 Schraudolph on VectorE:
# probs = bitcast_fp16(int16(lt*A + tabb)), tabb = round(A*rb + B) — one op,
# no separate bias multiply. Remaining tiles: exp on ScalarE + bias multiply
# on Pool (MULT_POOL_SET) or VectorE.
SCH_SET = frozenset(
    [(0, 0), (0, 3), (0, 5), (1, 1), (1, 3), (1, 4), (1, 6)]
)
MULT_POOL_SET = frozenset(
    [(0, 4), (1, 1), (1, 6)]
)
# fp16 Schraudolph: exp(x) ~= bitcast_fp16(int16(A*x + Bc))
SCH_A = float(1024.0 / np.log(2.0))
SCH_B = 15316.0

LAST_RESULTS = []
LAST_INMAPS = None


def _build_merged(repeat=1):
    nc = bacc.Bacc()
    qtb = nc.declare_dram_parameter("qtb", [D, S], BF16, isOutput=False)
    wqk = nc.declare_dram_parameter("wqk", [D, NHP * P], BF16, isOutput=False)
    wv = nc.declare_dram_parameter("wv", [D, NHP * HPC * KD], BF16, isOutput=False)
    bqk = nc.declare_dram_parameter("bqk", [P, NHP], F32, isOutput=False)
    sel = nc.declare_dram_parameter("sel", [4, 2 * P], BF16, isOutput=False)
    wo = nc.declare_dram_parameter("wo", [NH * KD, D], BF16, isOutput=False)
    # compact shifted bias tables: [p, hp, h, x]; tab = exp(rb) (bf16, for
    # the ScalarE-exp path's multiply), tabb = round(A*rb + B) (int16, the
    # fused addend for the Schraudolph path)
    tab = nc.declare_dram_parameter(
        "tab", [P, NHP * HPC * TABX], BF16, isOutput=False
    )
    tabb = nc.declare_dram_parameter(
        "tabb", [P, NHP * HPC * TABX], I16, isOutput=False
    )
    o = nc.declare_dram_parameter("o", [S, D], F32, isOutput=True)

    tab_r = tab.rearrange("p (hp h x) -> p hp h x", hp=NHP, h=HPC)
    tabb_r = tabb.rearrange("p (hp h x) -> p hp h x", hp=NHP, h=HPC)

    with tile.TileContext(nc) as tc:
        with (
            tc.tile_pool(name="const", bufs=1) as cpool,
            tc.tile_pool(name="qkp", bufs=2) as qkpool,
            tc.tile_pool(name="vp", bufs=2) as vpool,
            tc.tile_pool(name="repp", bufs=2) as reppool,
            tc.tile_pool(name="ebp", bufs=2) as ebpool,
            tc.tile_pool(name="expp", bufs=6) as exppool,
            tc.tile_pool(name="probsp", bufs=6) as probspool,
            tc.tile_pool(name="attp", bufs=16) as attpool,
            tc.tile_pool(name="zcp", bufs=1) as zcpool,
            tc.tile_pool(name="atnp", bufs=4) as atnpool,
            tc.tile_pool(name="atfp", bufs=1) as atfpool,
            tc.tile_pool(name="outp", bufs=2) as opool,
        ):
            # ---- constants ----
            qtb_t = cpool.tile([P, 4, S], BF16, name="qtb_t")
            nc.sync.dma_start(qtb_t, qtb.rearrange("(c p) s -> p c s", p=P))
            wqk_sb = cpool.tile([P, 4, NHP, P], BF16, name="wqk_sb")
            nc.sync.dma_start(
                wqk_sb, wqk.rearrange("(c p) (hp m) -> p c hp m", p=P, hp=NHP)
            )
            wv_sb = cpool.tile([P, 4, NHP, HPC * KD], BF16, name="wv_sb")
            nc.sync.dma_start(
                wv_sb, wv.rearrange("(c p) (hp m) -> p c hp m", p=P, hp=NHP)
            )
            bqk_sb = cpool.tile([P, NHP], F32, name="bqk_sb")
            nc.sync.dma_start(bqk_sb, bqk[:, :])
            sel_sb = cpool.tile([P, 2, P], BF16, name="sel_sb")
            wo_sb = cpool.tile([P, 4, D], BF16, name="wo_sb")
            tail_consts = []

            def load_tail_consts():
                # deferred so these DMAs don't compete with the startup loads
                if not tail_consts:
                    nc.sync.dma_start(
                        sel_sb[0:4], sel.rearrange("r (i p) -> r i p", i=2)
                    )
                    nc.sync.dma_start(wo_sb, wo.rearrange("(c p) d -> p c d", p=P))
                    tail_consts.append(True)

            zc_hp = {}

            # persistent 64-wide v tiles (double-buffered across hp):
            # cols 0:32 = V (rewritten per hp), col 32 = ones (Z rides
            # along), cols 33:64 = zeros so PV writes every atps row --
            # constant columns initialized once.
            v_bs = [
                vpool.tile([P, 8, HPC, 64], BF16, name=f"v_b{i}", tag=f"v_b{i}")
                for i in range(2)
            ]
            for vb in v_bs:
                nc.vector.memset(vb[:, :, :, KD + 1 :], 0.0)
                nc.vector.memset(vb[:, :, :, KD : KD + 1], 1.0)

            def emit_logits(nsp, g, rep, qk_b, tsl, ssl):
                for h in range(HPC):
                    if g == 0:
                        nc.tensor.matmul(
                            nsp["lt"][g][:, h * 512 : (h + 1) * 512],
                            lhsT=rep[h * KD : (h + 1) * KD, tsl],
                            rhs=qk_b[h * KD : (h + 1) * KD, ssl],
                            start=True,
                            stop=True,
                            tile_position=(h * KD, 0),
                        )
                    else:
                        nc.tensor.matmul(
                            nsp["lt"][g][:, h * 512 : (h + 1) * 512],
                            lhsT=qk_b[64 + h * KD : 64 + (h + 1) * KD, tsl],
                            rhs=rep[64 + h * KD : 64 + (h + 1) * KD, ssl],
                            start=True,
                            stop=True,
                            tile_position=(64 + h * KD, 0),
                        )

            def emit_sc(nsp, hp, sc, qk_b, rep, v_b, tab_t, tabb_t):
                ssl = slice(sc * 512, (sc + 1) * 512)
                atps = nsp["psattn"].tile([P, 512], F32, name="atps", tag="atps")
                for tj in range(8):
                    g = tj % 2
                    tsl = slice(tj * P, (tj + 1) * P)
                    with tc.high_priority(offset=64):
                        emit_logits(nsp, g, rep, qk_b, tsl, ssl)
                    ltg = nsp["lt"][g]
                    probs = probspool.tile([P, 1024], BF16, name="probs", tag="probs")
                    off = (31 - 4 * tj + 16 * sc) * 32
                    if (sc, tj) in SCH_SET:
                        # bias-fused Schraudolph: probs = fp16^(lt*A + tabb)
                        nc.vector.scalar_tensor_tensor(
                            probs.bitcast(I16).rearrange("p (h s) -> p h s", h=HPC),
                            ltg.rearrange("p (h s) -> p h s", h=HPC),
                            SCH_A,
                            tabb_t[:, :, off : off + 512],
                            mybir.AluOpType.mult,
                            mybir.AluOpType.add,
                        )
                    else:
                        exp_t = exppool.tile(
                            [P, 1024], BF16, name="exp_t", tag="exp_t"
                        )
                        nc.scalar.activation(
                            exp_t, ltg, mybir.ActivationFunctionType.Exp
                        )
                        mult_eng = (
                            nc.gpsimd if (sc, tj) in MULT_POOL_SET else nc.vector
                        )
                        mult_eng.tensor_tensor(
                            probs.rearrange("p (h s) -> p h s", h=HPC),
                            exp_t.rearrange("p (h s) -> p h s", h=HPC),
                            tab_t[:, :, off : off + 512],
                            mybir.AluOpType.mult,
                        )
                    for h in range(HPC):
                        nc.tensor.matmul(
                            atps[h * 64 : h * 64 + 64, :],
                            lhsT=v_b[:, tj, h, :],
                            rhs=probs[:, h * 512 : (h + 1) * 512],
                            start=(tj == 0),
                            stop=(tj == 7),
                            tile_position=(0, h * 64),
                            skip_group_check=True,
                        )
                au = attpool.tile([P, 512], BF16, name="au", tag="au")
                nc.scalar.copy(au, atps)
                pending_zc.append((hp, sc, au))
                return au

            pending_zc = []

            def flush_zc():
                while pending_zc:
                    hp_, sc_, au = pending_zc.pop(0)
                    if hp_ not in zc_hp:
                        zc_hp[hp_] = zcpool.tile(
                            [4, 512], BF16, name=f"zc{hp_}", tag="zc", bufs=3
                        )
                    z = zc_hp[hp_]
                    nc.sync.dma_start(z[2 * sc_ : 2 * sc_ + 1, :], au[32:33, :])
                    nc.sync.dma_start(
                        z[2 * sc_ + 1 : 2 * sc_ + 2, :], au[96:97, :]
                    )

            def emit_hp(nsp, hp, att_un, mid=None):
                flush_zc()
                qk_b = qkpool.tile([P, S], BF16, name="qk_b", tag="qk_b")
                v_b = v_bs[hp % 2]
                for sc in range(2):
                    ssl = slice(sc * 512, (sc + 1) * 512)
                    qkps = nsp["ps1"].tile(
                        [P, 512], F32, name="qkps", tag="proj", bufs=2
                    )
                    for ch in range(4):
                        nc.tensor.matmul(
                            qkps,
                            lhsT=wqk_sb[:, ch, hp, :],
                            rhs=qtb_t[:, ch, ssl],
                            start=(ch == 0),
                            stop=(ch == 3),
                        )
                    nc.scalar.add(qk_b[:, ssl], qkps, bqk_sb[:, hp : hp + 1])
                vps = nsp["ps1"].tile([P, 512], F32, name="vps", tag="proj", bufs=2)
                for tj in range(8):
                    for ch in range(4):
                        nc.tensor.matmul(
                            vps[:, tj * 64 : (tj + 1) * 64],
                            lhsT=qtb_t[:, ch, tj * P : (tj + 1) * P],
                            rhs=wv_sb[:, ch, hp, :],
                            start=(ch == 0),
                            stop=(ch == 3),
                        )
                nc.vector.tensor_copy(
                    v_b[:, :, :, 0:KD],
                    vps.rearrange("p (tj h k) -> p tj h k", tj=8, h=HPC),
                )
                rep = reppool.tile([P, S], BF16, name="rep", tag="rep")
                nc.sync.dma_start(rep[0:64, :], qk_b[64:128, :])
                nc.sync.dma_start(rep[64:128, :], qk_b[0:64, :])
                tab_t = ebpool.tile([P, HPC, TABX], BF16, name="tab_t", tag="tab_t")
                nc.sync.dma_start(tab_t, tab_r[:, hp])
                tabb_t = ebpool.tile(
                    [P, HPC, TABX], I16, name="tabb_t", tag="tabb_t"
                )
                nc.sync.dma_start(tabb_t, tabb_r[:, hp])
                if mid is not None:
                    mid()
                for sc in range(2):
                    att_un[(hp, sc)] = emit_sc(
                        nsp, hp, sc, qk_b, rep, v_b, tab_t, tabb_t
                    )

            def emit_norm_hp(hp, att_un, atf, rz_unused, rzpool):
                # 1/Z for this hp's 4 Z rows on VectorE, then per sc:
                # partition broadcast via a K=4 selection matmul, normalize,
                # scatter into the atf chunk for this hp.
                load_tail_consts()
                rz = zcpool.tile([4, 512], BF16, name=f"rz{hp}", tag="rz", bufs=2)
                with nc.allow_low_precision(reason="1/Z in fp16 (~5e-4)"):
                    nc.vector.reciprocal(rz[0:4, :], zc_hp[hp][0:4, :])
                for sc in range(2):
                    rzb = rzpool.tile([P, 512], F32, name="rzb", tag="rzb")
                    nc.tensor.matmul(
                        rzb,
                        lhsT=sel_sb[0:4, sc, :],
                        rhs=rz[0:4, :],
                        start=True,
                        stop=True,
                    )
                    au = att_un[(hp, sc)]
                    atn = atnpool.tile([P, 512], BF16, name="atn", tag="atn")
                    nc.vector.tensor_tensor(atn, au, rzb, mybir.AluOpType.mult)
                    # heads (2hp, 2hp+1) -> atf[hp%4] rows 64*(hp//4)+{0,32}
                    q, gq = hp % 4, hp // 4
                    for h in range(HPC):
                        nc.sync.dma_start(
                            atf[q][
                                64 * gq + 32 * h : 64 * gq + 32 * h + 32,
                                sc * 512 : sc * 512 + 512,
                            ],
                            atn[64 * h : 64 * h + 32, :],
                        )

            def emit_proj(atf, pso):
                for oc in range(2):
                    o_sb = opool.tile([P, 4, D], F32, name="o_sb", tag="o_sb")
                    for stl in range(4):
                        st = oc * 4 + stl
                        ps_o = pso.tile([P, D], F32, name="ps_o", tag="ps_o")
                        for ch in range(4):
                            nc.tensor.matmul(
                                ps_o,
                                lhsT=atf[ch][:, st * P : (st + 1) * P],
                                rhs=wo_sb[:, ch, :],
                                start=(ch == 0),
                                stop=(ch == 3),
                            )
                        nc.scalar.copy(o_sb[:, stl, :], ps_o)
                    nc.sync.dma_start(
                        o.rearrange("(oc st p) d -> p oc st d", p=P, oc=2)[
                            :, oc, :, :
                        ],
                        o_sb,
                    )

            for _rep in range(repeat):
                zc_hp.clear()
                del pending_zc[:]
                ps1_cm = tc.tile_pool(name="ps1", bufs=1, space="PSUM")
                ltpool_cm = tc.tile_pool(name="ltpool", bufs=1, space="PSUM")
                psattn_cm = tc.tile_pool(name="psattn", bufs=1, space="PSUM")
                nsp = {
                    "ps1": ps1_cm.__enter__(),
                    "ltpool": ltpool_cm.__enter__(),
                    "psattn": psattn_cm.__enter__(),
                }
                nsp["lt"] = [
                    nsp["ltpool"].tile([P, 1024], F32, name=f"lt{g}", tag=f"lt{g}")
                    for g in range(2)
                ]
                atf = [
                    atfpool.tile([P, S], BF16, name=f"atf{q}", tag=f"atf{q}")
                    for q in range(4)
                ]

                rzpool_cm = tc.tile_pool(name="rzps", bufs=1, space="PSUM")
                rzpool = rzpool_cm.__enter__()
                att_un = {}
                for hp in range(NHP):
                    mid = None
                    if hp >= 1:
                        mid = (lambda hp_: lambda: emit_norm_hp(
                            hp_, att_un, atf, None, rzpool
                        ))(hp - 1)
                    emit_hp(nsp, hp, att_un, mid=mid)
                flush_zc()
                emit_norm_hp(7, att_un, atf, None, rzpool)
                rzpool_cm.__exit__(None, None, None)
                psattn_cm.__exit__(None, None, None)
                ltpool_cm.__exit__(None, None, None)
                ps1_cm.__exit__(None, None, None)
                pso_cm = tc.tile_pool(name="pso", bufs=4, space="PSUM")
                pso = pso_cm.__enter__()
                emit_proj(atf, pso)
                pso_cm.__exit__(None, None, None)
    nc.compile()
    return nc


_NC = None
_IDX = None
_PREP_CACHE = {}


def _fingerprint(*arrs):
    import zlib
    h = 0
    for a in arrs:
        c = np.ascontiguousarray(a)
        h = zlib.crc32(c.view(np.uint8).reshape(-1), h)
        h = zlib.crc32(repr((c.shape, c.dtype.str)).encode(), h)
    return h


def _prep_static(Wq, Wk, Wv, Wo, bq, bk, rel_bias):
    scale = np.float32(KD ** -0.5)
    wqk_a = np.empty((D, NHP, 4, KD), dtype=np.float16)
    wv_a = np.empty((D, NHP, HPC, KD), dtype=np.float16)
    bqk_a = np.empty((P, NHP), dtype=np.float32)
    for hp in range(NHP):
        n0, n1 = 2 * hp, 2 * hp + 1
        wqk_a[:, hp, 0] = Wq[:, n0] * scale
        wqk_a[:, hp, 1] = Wq[:, n1] * scale
        wqk_a[:, hp, 2] = Wk[:, n0]
        wqk_a[:, hp, 3] = Wk[:, n1]
        wv_a[:, hp, 0] = Wv[:, n0]
        wv_a[:, hp, 1] = Wv[:, n1]
        bqk_a[:, hp] = np.concatenate(
            [bq[n0] * scale, bq[n1] * scale, bk[n0], bk[n1]]
        )
    # compact shifted bias table: tab[p=(bi,wt), n, x] =
    #   exp(rb[n, x//32 - bi, (x%32) - wt + 31]) (0 where d out of range).
    # The probs multiply for (tj, sc) then reads the contiguous slice
    # x in [off, off+512), off = (31 - 4*tj + 16*sc)*32.
    erb = np.exp(rel_bias).astype(np.float32)  # [N, 63, 63]
    brb = np.round(SCH_A * rel_bias + SCH_B).astype(np.int16)
    tab_a = np.zeros((P, NH, 63, 32), dtype=np.float16)
    tabb_a = np.zeros((P, NH, 63, 32), dtype=np.int16)
    bi_ = np.arange(P) // 32          # [P]
    wt_ = np.arange(P) % 32           # [P]
    ws_ = np.arange(32)
    for xb in range(63):
        d = xb - bi_                  # [P]
        valid = (d >= 0) & (d <= 62)
        wi = ws_[None, :] - wt_[:, None] + 31   # [P, 32] in [0, 62]
        for n in range(NH):
            blk = erb[n][np.clip(d, 0, 62)[:, None], wi]  # [P, 32]
            blk[~valid] = 0.0
            tab_a[:, n, xb, :] = blk
            blkb = brb[n][np.clip(d, 0, 62)[:, None], wi]
            blkb[~valid] = 0
            tabb_a[:, n, xb, :] = blkb
    tab_a = tab_a.reshape(P, NHP * HPC * TABX)
    tabb_a = tabb_a.reshape(P, NHP * HPC * TABX)
    sel_a = np.zeros((4, 2, P), dtype=np.float16)
    for sc in range(2):
        sel_a[2 * sc, sc, 0:64] = 1.0
        sel_a[2 * sc + 1, sc, 64:128] = 1.0
    sel_a = sel_a.reshape(4, 2 * P)
    # wo rows: atf[q] partition (gq*64 + h*32 + k) <-> head n = 2*(4*gq+q)+h
    wo_a = np.empty((4, 2, 2, KD, D), dtype=np.float16)
    for q in range(4):
        for gq in range(2):
            for h in range(2):
                wo_a[q, gq, h] = Wo[2 * (4 * gq + q) + h]
    return dict(
        wqk=np.ascontiguousarray(wqk_a.reshape(D, NHP * P)),
        wv=np.ascontiguousarray(wv_a.reshape(D, NHP * HPC * KD)),
        bqk=bqk_a,
        sel=sel_a,
        wo=np.ascontiguousarray(wo_a.reshape(NH * KD, D)),
        tab=np.ascontiguousarray(tab_a),
        tabb=np.ascontiguousarray(tabb_a),
    )


def kernel(query, Wq, bq, Wk, bk, Wv, bv, Wo, bo, rel_bias):
    global _NC
    query = np.asarray(query, dtype=np.float32)
    Wq = np.asarray(Wq, dtype=np.float32)
    Wk = np.asarray(Wk, dtype=np.float32)
    Wv = np.asarray(Wv, dtype=np.float32)
    Wo = np.asarray(Wo, dtype=np.float32)
    bq = np.asarray(bq, dtype=np.float32)
    bk = np.asarray(bk, dtype=np.float32)
    bv = np.asarray(bv, dtype=np.float32)
    bo = np.asarray(bo, dtype=np.float32)
    rel_bias = np.asarray(rel_bias, dtype=np.float32)

    trace = bool(int(os.environ.get("ATTN_TRACE", "0")))
    core_ids = list(range(NCORES))

    wkey = _fingerprint(Wq, Wk, Wv, Wo, bq, bk, rel_bias)
    if wkey not in _PREP_CACHE:
        _PREP_CACHE[wkey] = _prep_static(Wq, Wk, Wv, Wo, bq, bk, rel_bias)
    static_map = _PREP_CACHE[wkey]

    qkey = _fingerprint(query)
    if qkey not in _PREP_CACHE:
        _PREP_CACHE[qkey] = [
            np.ascontiguousarray(query[c].T.astype(np.float16)) for c in range(NCORES)
        ]
    qtbs = _PREP_CACHE[qkey]

    in_maps = [dict(qtb=qtbs[c], **static_map) for c in range(NCORES)]
    global LAST_INMAPS
    LAST_INMAPS = in_maps
    if _NC is None:
        _NC = _build_merged()
    r = run_bass_kernel_spmd(_NC, in_maps, core_ids, trace=trace)
    LAST_RESULTS.clear()
    LAST_RESULTS.append(r)

    out = np.stack([r.results[c]["o"] for c in range(NCORES)])  # [B, S, D]
    bo_eff = bo + np.einsum("nk,nkd->d", bv, Wo)
    return (out + bo_eff[None, None, :]).astype(np.float32)
